# revision 10
# baseline (speedup 1.0000x reference)
"""AdditiveAttention pooling kernel for 8 TRN2 NeuronCores.

reference:
    energy = tanh(lstm_output @ W_w.T + W_b)      # (B, S, H)
    scores = energy @ v_w                          # (B, S)
    scores = where(mask, scores, -1e9)
    weights = softmax(scores, axis=1)              # (B, S)
    context = einsum('bs,bsh->bh', weights, lstm_output)
    returns (context, weights)

Strategy: pure data-parallel over batch (B=64 -> 8 batches/core), no
collectives.  Single pass over x per core.  bf16 matmul inputs with fp32
PSUM accumulation.  |scores| <= ||v||_1 ~ 11.3 so softmax needs no
max-subtraction: w = exp(s + madd), Z = sum(w), out = w/Z.

Per batch (S=2048), processed in 2 half-batches of HT=1024 tokens:
  energy   psE[o_chunk] (128, 1024) += Wt[hc,oc].T @ xT[hc]   (8 MMs/oc)
  tanh     et[:, oc, :] = tanh(psE + bias[oc])                (1 long ACT/oc)
  score    psS[:, tc] += et[:, oc, 128tc:].T @ v[oc]          (32 MMs)
  mask     sc = psS + madd   (DVE)
  exp      w_stage[:, 8 cols] = exp(sc)                       (1 ACT)
  context  psC (1, 512) += w_col.T @ xn[tc]                   (8 MMs)
Epilogue per batch: Z = colsum(w_stage) via ones-matmul, 1/Z, scale, store.
"""

import sys

sys.path.insert(0, "/opt/trn_rl_repo")

import numpy as np
import ml_dtypes

import concourse.bass as bass
import concourse.tile as tile
from concourse import bacc, mybir
from concourse.bass_utils import run_bass_kernel_spmd

B, S, H = 64, 2048, 512
NCORES = 8
BPC = B // NCORES          # batches per core
HT = 1024                  # tokens per half-batch
NH = S // HT               # half-batches per batch (2)
NC = H // 128              # 128-sized chunks of H

bf16 = ml_dtypes.bfloat16
DT_BF = mybir.dt.bfloat16
DT_F32 = mybir.dt.float32

_CACHE = {}


def build(bpc=BPC, repeat=1):
    nc = bacc.Bacc(None, target_bir_lowering=False)

    # host-prearranged layouts (p = SBUF partition):
    #   xtp[b, p, hc, s] = x[b, s, hc*128+p]     (moving operand of energy MM)
    #   xnp[b, p, c, h]  = x[b, c*128+p, h]      (moving operand of ctx MM)
    xtp_d = nc.declare_dram_parameter("xtp", [bpc, 128, NC, S], DT_BF, isOutput=False)
    xnp_d = nc.declare_dram_parameter("xnp", [bpc, 128, S // 128, H], DT_BF, isOutput=False)
    wt_d = nc.declare_dram_parameter("wt", [NC, 128, H], DT_BF, isOutput=False)
    bias_d = nc.declare_dram_parameter("bias", [NC, 128], DT_F32, isOutput=False)
    vw_d = nc.declare_dram_parameter("vw", [NC, 128], DT_BF, isOutput=False)
    madd_d = nc.declare_dram_parameter("madd", [bpc, 128, S // 128], DT_F32, isOutput=False)
    ctx_d = nc.declare_dram_parameter("ctx", [bpc, H], DT_F32, isOutput=True)
    wts_d = nc.declare_dram_parameter("wts", [bpc, S], DT_F32, isOutput=True)

    TANH = mybir.ActivationFunctionType.Tanh
    EXP = mybir.ActivationFunctionType.Exp
    NCH = HT // 128  # token chunks per half-batch (8)

    with tile.TileContext(nc) as tc:
        with (
            tc.tile_pool(name="const", bufs=1) as cpool,
            tc.tile_pool(name="xt", bufs=3) as xtp,
            tc.tile_pool(name="xn", bufs=3) as xnp,
            tc.tile_pool(name="madd", bufs=2) as mdp,
            tc.tile_pool(name="et", bufs=2) as etp,
            tc.tile_pool(name="wstage", bufs=2) as wsp,
            tc.tile_pool(name="small", bufs=3) as smp,
            tc.tile_pool(name="out", bufs=2) as outp,
            tc.tile_pool(name="psE", bufs=2, space="PSUM") as psEp,   # 2 banks each
            tc.tile_pool(name="psS", bufs=2, space="PSUM") as psSp,   # 1 bank each
            tc.tile_pool(name="psC", bufs=2, space="PSUM") as psCp,   # 1 bank each
        ):
            # persistent constants
            wt_s = cpool.tile([128, NC, H], DT_BF)      # [h_p, hc, o]
            bias_s = cpool.tile([128, NC], DT_F32)      # [o_p, oc]
            v_s = cpool.tile([128, NC], DT_BF)          # [o_p, oc]
            ones_s = cpool.tile([128, 128], DT_BF)
            ones1_s = cpool.tile([1, 128], DT_F32)
            nc.sync.dma_start(wt_s[:], wt_d[:].rearrange("c p o -> p c o"))
            nc.sync.dma_start(bias_s[:], bias_d[:].rearrange("c p -> p c"))
            nc.sync.dma_start(v_s[:], vw_d[:].rearrange("c p -> p c"))
            nc.vector.memset(ones_s[:], 1.0)
            nc.vector.memset(ones1_s[:], 1.0)

            # Software pipeline: the score/exp/ctx stage of each half-batch is
            # emitted one half-batch later, with its 32 tiny score matmuls
            # zipped between the next half's energy matmuls so their weight
            # loads hide under the 213ns streaming matmuls.
            pend = None  # stage-2 of previous half: dict(sops, post, cops, epi)

            def emit_half(stage1_ops, tanh_ops):
                """Interleave this half's 32 energy MMs (+tanh after each oc
                group) with the previous half's deferred stage-2."""
                sops = pend["sops"] if pend else []
                zi = 0
                for i, e in enumerate(stage1_ops):
                    e()
                    if (i + 1) % (NC * NH) == 0:
                        tanh_ops[(i + 1) // (NC * NH) - 1]()
                    if i >= 8 and zi < len(sops):
                        sops[zi]()
                        zi += 1
                        if zi < len(sops):
                            sops[zi]()
                            zi += 1
                while zi < len(sops):
                    sops[zi]()
                    zi += 1
                if pend:
                    pend["post"]()
                    for c in pend["cops"]:
                        c()
                    if pend["epi"] is not None:
                        pend["epi"]()

            batches = [bb for _ in range(repeat) for bb in range(bpc)]
            for bi, b in enumerate(batches):
                w_stage = wsp.tile([128, S // 128], DT_BF)   # [t_p, chunk]
                psC = psCp.tile([1, H], DT_F32)
                md_t = mdp.tile([128, S // 128], DT_F32)     # [t_p, chunk]
                nc.sync.dma_start(md_t[:], madd_d[b])
                # full-batch loads: 16KB contiguous per partition on both sides
                xt_b = xtp.tile([128, NC, S], DT_BF)         # [h_p, hc, s]
                xn_b = xnp.tile([128, S // 128, H], DT_BF)   # [t_p, c, h]
                nc.sync.dma_start(xt_b[:], xtp_d[b])
                nc.gpsimd.dma_start(xn_b[:], xnp_d[b])
                for half in range(NH):
                    t0 = half * HT

                    et_h = etp.tile([128, NC, HT], DT_BF)    # [o_p, oc, t]
                    eops, tops = [], []
                    for oc in range(NC):
                        psE = psEp.tile([128, HT], DT_F32)
                        for hc in range(NC):
                            for jh in range(NH):
                                eops.append(
                                    (lambda pE=psE, o=oc, h=hc, j=jh: nc.tensor.matmul(
                                        pE[:, j * 512 : (j + 1) * 512],
                                        wt_s[:, h, o * 128 : (o + 1) * 128],
                                        xt_b[:, h, t0 + j * 512 : t0 + (j + 1) * 512],
                                        start=(h == 0),
                                        stop=(h == NC - 1),
                                    ))
                                )
                        tops.append(
                            (lambda pE=psE, o=oc, e=et_h: nc.scalar.activation(
                                e[:, o, :], pE[:], TANH, bias=bias_s[:, o : o + 1]
                            ))
                        )
                    emit_half(eops, tops)

                    # deferred stage-2 for this half
                    psS = psSp.tile([128, NCH], DT_F32)
                    sops = []
                    for t in range(NCH):
                        for oc in range(NC):
                            sops.append(
                                (lambda pS=psS, e=et_h, t_=t, o=oc: nc.tensor.matmul(
                                    pS[:, t_ : t_ + 1],
                                    e[:, o, t_ * 128 : (t_ + 1) * 128],
                                    v_s[:, o : o + 1],
                                    start=(o == 0),
                                    stop=(o == NC - 1),
                                ))
                            )

                    def post(pS=psS, ws=w_stage, md=md_t, h=half):
                        sc_m = smp.tile([128, NCH], DT_F32)
                        nc.vector.tensor_add(
                            sc_m[:], pS[:], md[:, h * NCH : (h + 1) * NCH]
                        )
                        nc.scalar.activation(
                            ws[:, h * NCH : (h + 1) * NCH], sc_m[:], EXP
                        )

                    cops = []
                    for t in range(NCH):
                        c = half * NCH + t
                        cops.append(
                            (lambda ws=w_stage, xn=xn_b, c_=c, pC=psC: nc.tensor.matmul(
                                pC[:],
                                ws[:, c_ : c_ + 1],
                                xn[:, c_, :],
                                start=(c_ == 0),
                                stop=(c_ == S // 128 - 1),
                            ))
                        )

                    epi = None
                    if half == NH - 1:
                        def epi(ws=w_stage, pC=psC, b_=b):
                            psZ = psSp.tile([1, S // 128], DT_F32, tag="psS")
                            nc.tensor.matmul(
                                psZ[:], ones_s[:, 0:1], ws[:], start=True, stop=True
                            )
                            z1 = smp.tile([1, 1], DT_F32)
                            nc.vector.tensor_reduce(
                                z1[:], psZ[:],
                                axis=mybir.AxisListType.X, op=mybir.AluOpType.add,
                            )
                            psZb = psSp.tile([128, 1], DT_F32, tag="psS")
                            nc.tensor.matmul(
                                psZb[:], ones1_s[:], z1[:], start=True, stop=True
                            )
                            rz = smp.tile([128, 1], DT_F32)
                            nc.vector.reciprocal(rz[:], psZb[:])
                            wout = outp.tile([128, S // 128], DT_F32)
                            nc.vector.tensor_scalar_mul(wout[:], ws[:], rz[:])
                            ctxout = outp.tile([1, H], DT_F32)
                            nc.vector.tensor_scalar_mul(ctxout[:], pC[:], rz[0:1, :])
                            nc.sync.dma_start(
                                wts_d[b_].rearrange("(c p) -> p c", p=128), wout[:]
                            )
                            nc.sync.dma_start(ctx_d[b_ : b_ + 1, :], ctxout[:])

                    pend = {"sops": sops, "post": post, "cops": cops, "epi": epi}

            # flush the last half-batch's stage-2
            emit_half([], [])

    nc.compile()
    return nc


def _prep_inputs(lstm_output, mask, W_w, W_b, v_w):
    x = np.asarray(lstm_output, dtype=np.float32)
    xb = x.astype(bf16)                                   # (B, S, H)
    # xtp[b, p, hc, s] = x[b, s, hc*128+p]
    xtp = np.ascontiguousarray(
        xb.reshape(B, S, NC, 128).transpose(0, 3, 2, 1)
    )
    # xnp[b, p, c, h] = x[b, c*128+p, h]
    xnp = np.ascontiguousarray(
        xb.reshape(B, S // 128, 128, H).transpose(0, 2, 1, 3)
    )
    wt = np.ascontiguousarray(np.asarray(W_w, np.float32).T.reshape(NC, 128, H)).astype(bf16)
    biasc = np.ascontiguousarray(np.asarray(W_b, np.float32).reshape(NC, 128))
    vwc = np.ascontiguousarray(np.asarray(v_w, np.float32).reshape(NC, 128)).astype(bf16)
    madd = np.where(np.asarray(mask), np.float32(0.0), np.float32(-1e9)).astype(np.float32)
    # madd_d[b, p, c] = madd[b, c*128+p]
    madd = np.ascontiguousarray(madd.reshape(B, S // 128, 128).transpose(0, 2, 1))

    in_maps = []
    for c in range(NCORES):
        sl = slice(c * BPC, (c + 1) * BPC)
        in_maps.append(
            {
                "xtp": np.ascontiguousarray(xtp[sl]),
                "xnp": np.ascontiguousarray(xnp[sl]),
                "wt": wt,
                "bias": biasc,
                "vw": vwc,
                "madd": np.ascontiguousarray(madd[sl]),
            }
        )
    return in_maps


def kernel(lstm_output, mask, W_w, W_b, v_w):
    if "nc" not in _CACHE:
        _CACHE["nc"] = build()
    nc = _CACHE["nc"]
    in_maps = _prep_inputs(lstm_output, mask, W_w, W_b, v_w)
    res = run_bass_kernel_spmd(nc, in_maps, core_ids=list(range(NCORES)))
    ctx = np.concatenate([res.results[i]["ctx"] for i in range(NCORES)], axis=0)
    wts = np.concatenate([res.results[i]["wts"] for i in range(NCORES)], axis=0)
    return ctx.astype(np.float32), wts.astype(np.float32)


# revision 11
# speedup vs baseline: 1.1060x; 1.1060x over previous
"""AdditiveAttention pooling kernel for 8 TRN2 NeuronCores.

reference:
    energy = tanh(lstm_output @ W_w.T + W_b)      # (B, S, H)
    scores = energy @ v_w                          # (B, S)
    scores = where(mask, scores, -1e9)
    weights = softmax(scores, axis=1)              # (B, S)
    context = einsum('bs,bsh->bh', weights, lstm_output)
    returns (context, weights)

Strategy: pure data-parallel over batch (B=64 -> 8 batches/core), no
collectives.  Single pass over x per core.  bf16 matmul inputs with fp32
PSUM accumulation.  |scores| <= ||v||_1 ~ 11.3 so softmax needs no
max-subtraction: w = exp(s + madd), Z = sum(w), out = w/Z.

Per batch (S=2048), processed in 2 half-batches of HT=1024 tokens:
  energy   psE[o_chunk] (128, 1024) += Wt[hc,oc].T @ xT[hc]   (8 MMs/oc)
  tanh     et[:, oc, :] = tanh(psE + bias[oc])                (1 long ACT/oc)
  score    psS[:, tc] += et[:, oc, 128tc:].T @ v[oc]          (32 MMs)
  mask     sc = psS + madd   (DVE)
  exp      w_stage[:, 8 cols] = exp(sc)                       (1 ACT)
  context  psC (1, 512) += w_col.T @ xn[tc]                   (8 MMs)
Epilogue per batch: Z = colsum(w_stage) via ones-matmul, 1/Z, scale, store.
"""

import sys

sys.path.insert(0, "/opt/trn_rl_repo")

import numpy as np
import ml_dtypes

import concourse.bass as bass
import concourse.tile as tile
from concourse import bacc, mybir
from concourse.bass_utils import run_bass_kernel_spmd

B, S, H = 64, 2048, 512
NCORES = 8
BPC = B // NCORES          # batches per core
HT = 1024                  # tokens per half-batch
NH = S // HT               # half-batches per batch (2)
NC = H // 128              # 128-sized chunks of H

bf16 = ml_dtypes.bfloat16
DT_BF = mybir.dt.bfloat16
DT_F32 = mybir.dt.float32

_CACHE = {}


def build(bpc=BPC, repeat=1):
    nc = bacc.Bacc(None, target_bir_lowering=False)

    # host-prearranged layouts (p = SBUF partition):
    #   xtp[b, p, hc, s] = x[b, s, hc*128+p]     (moving operand of energy MM)
    #   xnp[b, p, c, h]  = x[b, c*128+p, h]      (moving operand of ctx MM)
    xtp_d = nc.declare_dram_parameter("xtp", [bpc, 128, NC, S], DT_BF, isOutput=False)
    xnp_d = nc.declare_dram_parameter("xnp", [bpc, 128, S // 128, H], DT_BF, isOutput=False)
    wt_d = nc.declare_dram_parameter("wt", [NC, 128, H], DT_BF, isOutput=False)
    bias_d = nc.declare_dram_parameter("bias", [NC, 128], DT_F32, isOutput=False)
    vw_d = nc.declare_dram_parameter("vw", [NC, 128], DT_BF, isOutput=False)
    madd_d = nc.declare_dram_parameter("madd", [bpc, 128, S // 128], DT_F32, isOutput=False)
    ctx_d = nc.declare_dram_parameter("ctx", [bpc, H], DT_F32, isOutput=True)
    wts_d = nc.declare_dram_parameter("wts", [bpc, S], DT_F32, isOutput=True)

    TANH = mybir.ActivationFunctionType.Tanh
    EXP = mybir.ActivationFunctionType.Exp
    NCH = HT // 128  # token chunks per half-batch (8)

    with tile.TileContext(nc) as tc:
        with (
            tc.tile_pool(name="const", bufs=1) as cpool,
            tc.tile_pool(name="xt", bufs=3) as xtp,
            tc.tile_pool(name="xn", bufs=3) as xnp,
            tc.tile_pool(name="madd", bufs=2) as mdp,
            tc.tile_pool(name="et", bufs=2) as etp,
            tc.tile_pool(name="wstage", bufs=2) as wsp,
            tc.tile_pool(name="small", bufs=3) as smp,
            tc.tile_pool(name="out", bufs=2) as outp,
            tc.tile_pool(name="psE", bufs=3, space="PSUM") as psEp,   # 2 banks each
            tc.tile_pool(name="psS", bufs=1, space="PSUM") as psSp,   # 1 bank each
            tc.tile_pool(name="psC", bufs=1, space="PSUM") as psCp,   # 1 bank each
        ):
            # persistent constants
            wt_s = cpool.tile([128, NC, H], DT_BF)      # [h_p, hc, o]
            bias_s = cpool.tile([128, NC], DT_F32)      # [o_p, oc]
            v_s = cpool.tile([128, NC], DT_BF)          # [o_p, oc]
            ones_s = cpool.tile([128, 128], DT_BF)
            ones1_s = cpool.tile([1, 128], DT_F32)
            nc.sync.dma_start(wt_s[:], wt_d[:].rearrange("c p o -> p c o"))
            nc.sync.dma_start(bias_s[:], bias_d[:].rearrange("c p -> p c"))
            nc.sync.dma_start(v_s[:], vw_d[:].rearrange("c p -> p c"))
            nc.vector.memset(ones_s[:], 1.0)
            nc.vector.memset(ones1_s[:], 1.0)

            # Software pipeline: the score/exp/ctx stage of each half-batch is
            # emitted one half-batch later, with its 32 tiny score matmuls
            # zipped between the next half's energy matmuls so their weight
            # loads hide under the 213ns streaming matmuls.
            pend = None  # stage-2 of previous half: dict(sops, post, cops, epi)

            def emit_half(stage1_ops, tanh_ops):
                """Emit this half's energy MMs (+tanh after each oc group),
                then the previous half's deferred stage-2 as a block."""
                for i, e in enumerate(stage1_ops):
                    e()
                    if (i + 1) % (NC * NH) == 0:
                        tanh_ops[(i + 1) // (NC * NH) - 1]()
                if pend:
                    for s in pend["sops"]:
                        s()
                    pend["post"]()
                    for c in pend["cops"]:
                        c()
                    if pend["epi"] is not None:
                        pend["epi"]()

            batches = [bb for _ in range(repeat) for bb in range(bpc)]
            for bi, b in enumerate(batches):
                w_stage = wsp.tile([128, S // 128], DT_BF)   # [t_p, chunk]
                psC = psCp.tile([1, H], DT_F32)
                md_t = mdp.tile([128, S // 128], DT_F32)     # [t_p, chunk]
                nc.sync.dma_start(md_t[:], madd_d[b])
                for half in range(NH):
                    t0 = half * HT
                    xt_b = xtp.tile([128, NC, HT], DT_BF)    # [h_p, hc, t]
                    xn_b = xnp.tile([128, NCH, H], DT_BF)    # [t_p, tc, h]
                    nc.sync.dma_start(
                        xt_b[:], xtp_d[b, :, :, t0 : t0 + HT]
                    )
                    nc.gpsimd.dma_start(
                        xn_b[:], xnp_d[b, :, half * NCH : (half + 1) * NCH, :]
                    )

                    et_h = etp.tile([128, NC, HT], DT_BF)    # [o_p, oc, t]
                    eops, tops = [], []
                    for oc in range(NC):
                        psE = psEp.tile([128, HT], DT_F32)
                        for hc in range(NC):
                            for jh in range(NH):
                                eops.append(
                                    (lambda pE=psE, o=oc, h=hc, j=jh: nc.tensor.matmul(
                                        pE[:, j * 512 : (j + 1) * 512],
                                        wt_s[:, h, o * 128 : (o + 1) * 128],
                                        xt_b[:, h, j * 512 : (j + 1) * 512],
                                        start=(h == 0),
                                        stop=(h == NC - 1),
                                    ))
                                )
                        tops.append(
                            (lambda pE=psE, o=oc, e=et_h: nc.scalar.activation(
                                e[:, o, :], pE[:], TANH, bias=bias_s[:, o : o + 1]
                            ))
                        )
                    emit_half(eops, tops)

                    # deferred stage-2 for this half
                    psS = psSp.tile([128, NCH], DT_F32)
                    sops = []
                    for t in range(NCH):
                        for oc in range(NC):
                            sops.append(
                                (lambda pS=psS, e=et_h, t_=t, o=oc: nc.tensor.matmul(
                                    pS[:, t_ : t_ + 1],
                                    e[:, o, t_ * 128 : (t_ + 1) * 128],
                                    v_s[:, o : o + 1],
                                    start=(o == 0),
                                    stop=(o == NC - 1),
                                ))
                            )

                    def post(pS=psS, ws=w_stage, md=md_t, h=half):
                        sc_m = smp.tile([128, NCH], DT_F32)
                        nc.vector.tensor_add(
                            sc_m[:], pS[:], md[:, h * NCH : (h + 1) * NCH]
                        )
                        nc.scalar.activation(
                            ws[:, h * NCH : (h + 1) * NCH], sc_m[:], EXP
                        )

                    cops = []
                    for t in range(NCH):
                        c = half * NCH + t
                        cops.append(
                            (lambda ws=w_stage, xn=xn_b, c_=c, t_=t, pC=psC: nc.tensor.matmul(
                                pC[:],
                                ws[:, c_ : c_ + 1],
                                xn[:, t_, :],
                                start=(c_ == 0),
                                stop=(c_ == S // 128 - 1),
                            ))
                        )

                    epi = None
                    if half == NH - 1:
                        def epi(ws=w_stage, pC=psC, b_=b):
                            psZ = psSp.tile([1, S // 128], DT_F32, tag="psS")
                            nc.tensor.matmul(
                                psZ[:], ones_s[:, 0:1], ws[:], start=True, stop=True
                            )
                            z1 = smp.tile([1, 1], DT_F32)
                            nc.vector.tensor_reduce(
                                z1[:], psZ[:],
                                axis=mybir.AxisListType.X, op=mybir.AluOpType.add,
                            )
                            psZb = psSp.tile([128, 1], DT_F32, tag="psS")
                            nc.tensor.matmul(
                                psZb[:], ones1_s[:], z1[:], start=True, stop=True
                            )
                            rz = smp.tile([128, 1], DT_F32)
                            nc.vector.reciprocal(rz[:], psZb[:])
                            wout = outp.tile([128, S // 128], DT_F32)
                            nc.vector.tensor_scalar_mul(wout[:], ws[:], rz[:])
                            ctxout = outp.tile([1, H], DT_F32)
                            nc.vector.tensor_scalar_mul(ctxout[:], pC[:], rz[0:1, :])
                            nc.sync.dma_start(
                                wts_d[b_].rearrange("(c p) -> p c", p=128), wout[:]
                            )
                            nc.sync.dma_start(ctx_d[b_ : b_ + 1, :], ctxout[:])

                    pend = {"sops": sops, "post": post, "cops": cops, "epi": epi}

            # flush the last half-batch's stage-2
            emit_half([], [])

    nc.compile()
    return nc


def _prep_inputs(lstm_output, mask, W_w, W_b, v_w):
    x = np.asarray(lstm_output, dtype=np.float32)
    xb = x.astype(bf16)                                   # (B, S, H)
    # xtp[b, p, hc, s] = x[b, s, hc*128+p]
    xtp = np.ascontiguousarray(
        xb.reshape(B, S, NC, 128).transpose(0, 3, 2, 1)
    )
    # xnp[b, p, c, h] = x[b, c*128+p, h]
    xnp = np.ascontiguousarray(
        xb.reshape(B, S // 128, 128, H).transpose(0, 2, 1, 3)
    )
    wt = np.ascontiguousarray(np.asarray(W_w, np.float32).T.reshape(NC, 128, H)).astype(bf16)
    biasc = np.ascontiguousarray(np.asarray(W_b, np.float32).reshape(NC, 128))
    vwc = np.ascontiguousarray(np.asarray(v_w, np.float32).reshape(NC, 128)).astype(bf16)
    madd = np.where(np.asarray(mask), np.float32(0.0), np.float32(-1e9)).astype(np.float32)
    # madd_d[b, p, c] = madd[b, c*128+p]
    madd = np.ascontiguousarray(madd.reshape(B, S // 128, 128).transpose(0, 2, 1))

    in_maps = []
    for c in range(NCORES):
        sl = slice(c * BPC, (c + 1) * BPC)
        in_maps.append(
            {
                "xtp": np.ascontiguousarray(xtp[sl]),
                "xnp": np.ascontiguousarray(xnp[sl]),
                "wt": wt,
                "bias": biasc,
                "vw": vwc,
                "madd": np.ascontiguousarray(madd[sl]),
            }
        )
    return in_maps


def kernel(lstm_output, mask, W_w, W_b, v_w):
    if "nc" not in _CACHE:
        _CACHE["nc"] = build()
    nc = _CACHE["nc"]
    in_maps = _prep_inputs(lstm_output, mask, W_w, W_b, v_w)
    res = run_bass_kernel_spmd(nc, in_maps, core_ids=list(range(NCORES)))
    ctx = np.concatenate([res.results[i]["ctx"] for i in range(NCORES)], axis=0)
    wts = np.concatenate([res.results[i]["wts"] for i in range(NCORES)], axis=0)
    return ctx.astype(np.float32), wts.astype(np.float32)
